# revision 4
# baseline (speedup 1.0000x reference)
"""Distributed Trainium2 kernel for nn_Attention_335007449342.

Head-parallel tensor parallelism over 8 NeuronCores with a
sequence-parallel switch before the output projection:
  - LoRA adapters are folded into the dense weights on the host
    (W_eff = W + lora2 @ lora1, exact by linearity); the attention
    scale 1/sqrt(HD) is folded into wq_eff; tanh(gate)*new_gate is
    folded into the adapter V projection; the tiny adapter K/V
    projections (plain wk/wv per the reference) are computed host-side.
  - each core owns 4 heads (512 of 4096 qkv dims). Phase 1 computes
    Q^T/K^T (dims x tokens) and V (tokens x dims) with 512-wide
    matmuls (weight loads hide under the 512-cycle moving operand);
    batch-0 Q/K stay in SBUF, batch-1 Q/K and both batches' V
    roundtrip through DRAM and reload per-head during attention.
  - attention per (query-block, head) unit runs fully transposed:
    scores^T = K_chunk^T @ Q (128x512), exp on scalar engine,
    softmax denominators via ones-vector matmuls, PV as
    out^T = V_chunk^T @ P^T (N=512 -- no PE transposes anywhere),
    adapter cross-attention in [AL, q] layout accumulated into the
    same PSUM, normalization by 1/s as a per-column broadcast
    multiply on the vector engine.
  - per batch, attention outputs (out^T) are exchanged with a bf16
    AllToAll (2 MB per core) so core c ends with all 4096 dims for
    256 tokens of the batch, then projects locally with wo_eff
    streamed from HBM; batch-0 projection interleaves with batch-1
    attention (emitted ahead of each unit so the in-order PE queue
    is not coupled to the scalar-engine exp pace).

All matmuls run in bf16 (fp32 PSUM accumulation); softmax in fp32
without max-subtraction (scores are O(10), mask -1e9 underflows exp
to 0).
"""

import math
import sys

sys.path.insert(0, "/opt/trn_rl_repo")

import numpy as np
import ml_dtypes

B, S, D, H, HD, AL, R = 2, 2048, 4096, 32, 128, 10, 16
NCORES = 8
HPC = H // NCORES          # 4 heads per core
LD = HPC * HD              # 512 local qkv dims per core
T = B * S                  # 4096 tokens
TB = 512                   # token block / query row-block
NTB = T // TB              # 8
KC = D // 128              # 32 contraction chunks over D
SKC = S // 128             # 16 key chunks per batch
TPC = S // NCORES          # 256 tokens per core per batch (a2a shard)
NQR = S // TB              # 4 query row-blocks per batch
SCALE = 1.0 / math.sqrt(HD)
BF16 = ml_dtypes.bfloat16

_CACHE = {}


def _mask_pattern(mask_np):
    """Per (qr query block of 512, kc key chunk of 128): classify the mask.
    keep=False when the whole block is ~-inf (softmax weight 0 -> skip),
    need_mask=True when the block has any nonzero mask value."""
    m = np.asarray(mask_np, np.float32)[0, 0]
    keep, need = [], []
    slots, reps, mmap = {}, [], {}
    for qr in range(NQR):
        krow, nrow = [], []
        for kc in range(SKC):
            blk = m[qr * TB:(qr + 1) * TB, kc * 128:(kc + 1) * 128].T
            krow.append(not bool((blk <= -1e8).all()))
            nrow.append(bool((blk != 0.0).any()) and krow[-1])
            if nrow[-1]:
                key = blk.astype(BF16).tobytes()
                if key not in slots:
                    slots[key] = len(reps)
                    reps.append((qr, kc))
                mmap[(qr, kc)] = slots[key]
        keep.append(tuple(krow))
        need.append(tuple(nrow))
    return tuple(keep), tuple(need), tuple(reps), tuple(sorted(mmap.items()))


def _build(keep=None, need_mask=None, mreps=None, mmap=None):
    import concourse.bass as bass
    import concourse.mybir as mybir
    import concourse.tile as tile
    from concourse import bacc

    f32 = mybir.dt.float32
    bf16 = mybir.dt.bfloat16
    AF = mybir.ActivationFunctionType
    if keep is None:
        keep = tuple((True,) * SKC for _ in range(NQR))
    if need_mask is None:
        need_mask = keep
    if mreps is None:
        mreps = tuple((qr, k) for qr in range(NQR) for k in range(SKC)
                      if need_mask[qr][k])
        mmap = tuple(((qr, k), i) for i, (qr, k) in enumerate(mreps))
    mslot = dict(mmap)

    nc = bacc.Bacc(None, target_bir_lowering=False, debug=True)

    xt = nc.declare_dram_parameter("xt", [D, T], bf16, isOutput=False)
    wqkvt = nc.declare_dram_parameter("wqkvt", [D, 3 * LD], bf16, isOutput=False)
    wot = nc.declare_dram_parameter("wot", [D, D], bf16, isOutput=False)
    aktp = nc.declare_dram_parameter("aktp", [128, HPC * B * AL], bf16, isOutput=False)
    avp = nc.declare_dram_parameter("avp", [B * AL, LD], bf16, isOutput=False)
    maskt = nc.declare_dram_parameter("maskt", [S, S], bf16, isOutput=False)
    out = nc.declare_dram_parameter("out", [B * TPC, D], f32, isOutput=True)

    rg8 = [list(range(NCORES))]
    # batch-0 runs h-major (head h's last use is unit 4h+3, so batch-1's
    # per-head reloads of the shared K/Q/V tiles can start early);
    # batch-1 runs qr-major (order is free there)
    units_h = [(qr, h) for h in range(HPC) for qr in range(NQR)]
    units_q = [(qr, h) for qr in range(NQR) for h in range(HPC)]

    with tile.TileContext(nc) as tc:
        with tc.tile_pool(name="dram", bufs=1, space="DRAM") as dram, \
             tc.tile_pool(name="persist", bufs=1) as persist:
            qt_d = dram.tile([LD, S], bf16)          # batch-1 roundtrip
            kt_d = dram.tile([LD, S], bf16)
            v_d = dram.tile([B * S, LD], bf16)       # both batches
            a2a_in = [dram.tile([NCORES * LD, TPC], bf16, name=f"a2ain{b}")
                      for b in range(B)]
            a2a_out = [dram.tile([NCORES * LD, TPC], bf16, name=f"a2aout{b}")
                       for b in range(B)]

            aktsb = persist.tile([128, HPC, B * AL], bf16)
            nc.scalar.dma_start(aktsb, aktp[:].rearrange("p (m a) -> p m a", m=HPC))
            avsb = [persist.tile([AL, LD], bf16, name=f"avsb{b}") for b in range(B)]
            for b in range(B):
                nc.scalar.dma_start(avsb[b], avp[b * AL:(b + 1) * AL, :])
            ones128 = persist.tile([128, 1], bf16)
            nc.vector.memset(ones128, 1.0)
            ones10 = persist.tile([AL, 1], bf16)
            nc.vector.memset(ones10, 1.0)

            # mask chunks (shared by both batches, deduped by content)
            msb = None
            if 0 < len(mreps) <= 8:
                msb = persist.tile([128, len(mreps), TB], bf16)
                mre = maskt[:].rearrange("(kc p) q -> p kc q", p=128)
                for i, (qr, k) in enumerate(mreps):
                    nc.sync.dma_start(msb[:, i, :],
                                      mre[:, k, qr * TB:(qr + 1) * TB])

            with tc.tile_pool(name="kqv", bufs=1) as kqv:
                ktsb = kqv.tile([128, HPC, S], bf16)
                qsb = kqv.tile([128, HPC, S], bf16)

                # ---------------- Phase 1: QKV projections ----------------
                with tc.tile_pool(name="wpool", bufs=1) as wpool, \
                     tc.tile_pool(name="xpool", bufs=2) as xpool, \
                     tc.tile_pool(name="spool", bufs=2) as spool, \
                     tc.tile_pool(name="qkps", bufs=3, space="PSUM") as qkps, \
                     tc.tile_pool(name="vps", bufs=2, space="PSUM") as vps:
                    wsb = wpool.tile([128, KC, 3 * LD], bf16)
                    wre = wqkvt[:].rearrange("(kc p) m -> p kc m", p=128)
                    for kg in range(8):
                        nc.sync.dma_start(wsb[:, kg * 4:(kg + 1) * 4, :],
                                          wre[:, kg * 4:(kg + 1) * 4, :])

                    xre = xt[:].rearrange("(kc p) t -> p kc t", p=128)
                    for tb in range(NTB):
                        b = tb // NQR
                        ts = (tb % NQR) * TB
                        t0 = tb * TB
                        xsb = xpool.tile([128, KC, TB], bf16, tag="x")
                        for xg in range(8):
                            nc.gpsimd.dma_start(
                                xsb[:, xg * 4:(xg + 1) * 4, :],
                                xre[:, xg * 4:(xg + 1) * 4, t0:t0 + TB])
                        # q^T and k^T tiles ([outdim, tok]), N=512 moving
                        for m in range(2 * HPC):
                            ps = qkps.tile([128, TB], f32, tag="ps")
                            for k in range(KC):
                                nc.tensor.matmul(
                                    ps, wsb[:, k, m * 128:(m + 1) * 128],
                                    xsb[:, k, :],
                                    start=(k == 0), stop=(k == KC - 1))
                            h = m % HPC
                            if b == 0:
                                dst = qsb if m < HPC else ktsb
                                if m % 2 == 0:
                                    nc.scalar.activation(
                                        dst[:, h, ts:ts + TB], ps, AF.Copy)
                                else:
                                    nc.vector.tensor_copy(
                                        dst[:, h, ts:ts + TB], ps)
                            else:
                                osb = spool.tile([128, TB], bf16, tag="qk")
                                if m % 2 == 0:
                                    nc.scalar.activation(osb, ps, AF.Copy)
                                else:
                                    nc.vector.tensor_copy(osb, ps)
                                dst_d = qt_d if m < HPC else kt_d
                                nc.sync.dma_start(
                                    dst_d[h * 128:(h + 1) * 128, ts:ts + TB],
                                    osb)
                        # v tiles in [token, dim] layout -> DRAM (both batches)
                        for tt in range(TB // 128):
                            ps = vps.tile([128, LD], f32, tag="pv")
                            for k in range(KC):
                                nc.tensor.matmul(
                                    ps, xsb[:, k, tt * 128:(tt + 1) * 128],
                                    wsb[:, k, 2 * LD:3 * LD],
                                    start=(k == 0), stop=(k == KC - 1))
                            vsb = spool.tile([128, LD], bf16, tag="qk")
                            if tt % 2 == 0:
                                nc.vector.tensor_copy(vsb, ps)
                            else:
                                nc.scalar.activation(vsb, ps, AF.Copy)
                            nc.sync.dma_start(
                                v_d[b * S + ts + tt * 128:
                                    b * S + ts + (tt + 1) * 128, :], vsb)

                # ---------------- Attention phases ----------------
                with tc.tile_pool(name="vpool", bufs=1) as vpool:
                    vasb = vpool.tile([128, SKC, HPC, HD], bf16)

                    def load_v_head(b, h):
                        nc.scalar.dma_start(
                            vasb[:, :, h, :],
                            v_d[b * S:(b + 1) * S, h * 128:(h + 1) * 128]
                            .rearrange("(kc p) d -> p kc d", p=128))

                    def load_b1_head(h):
                        nc.scalar.dma_start(
                            ktsb[:, h, :], kt_d[h * 128:(h + 1) * 128, :])
                        nc.scalar.dma_start(
                            qsb[:, h, :], qt_d[h * 128:(h + 1) * 128, :])
                        load_v_head(1, h)

                    def attn_batch(b, units, sps, ops, cps,
                                   ppool, apool, zpool,
                                   pre_unit=None, post_unit=None):
                        """Emit attention for one batch, 2-deep software
                        pipeline. pre_unit(i) emits extra PE work ahead of
                        unit i's scores; post_unit(i) after unit i's tail."""

                        def stage_scores(u):
                            qr, h = u
                            kept = [k for k in range(SKC) if keep[qr][k]]
                            qv = qsb[:, h, qr * TB:(qr + 1) * TB]
                            # PE outputs must land at base partition 0/32/64:
                            # row 0 = main softmax sum, rows 32.. = adapter
                            # scores, row 64 = adapter sum -- one PSUM bank
                            combo = cps.tile([65, TB], f32, tag="combo")
                            nc.tensor.matmul(
                                combo[32:32 + AL, :],
                                aktsb[:, h, b * AL:(b + 1) * AL], qv,
                                start=True, stop=True, skip_group_check=True)
                            ae2 = apool.tile([AL, TB], bf16, tag="ae")
                            nc.scalar.activation(ae2, combo[32:32 + AL, :], AF.Exp)
                            ptsb = ppool.tile([128, SKC, TB], bf16, tag="p")
                            for k in kept:
                                pss = sps.tile([128, TB], f32, tag="s")
                                nc.tensor.matmul(
                                    pss, ktsb[:, h, k * 128:(k + 1) * 128],
                                    qv, start=True, stop=True)
                                if need_mask[qr][k]:
                                    nc.vector.tensor_add(
                                        pss, pss, msb[:, mslot[(qr, k)], :])
                                nc.scalar.activation(ptsb[:, k, :], pss, AF.Exp)
                            return ptsb, ae2, combo

                        def stage_tail(u, ptsb, ae2, combo):
                            qr, h = u
                            kept = [k for k in range(SKC) if keep[qr][k]]
                            for i, k in enumerate(kept):
                                nc.tensor.matmul(
                                    combo[0:1, :], ones128, ptsb[:, k, :],
                                    start=(i == 0), stop=(i == len(kept) - 1),
                                    skip_group_check=True)
                            nc.tensor.matmul(
                                combo[64:65, :], ones10, ae2,
                                start=True, stop=True, skip_group_check=True)
                            rec_s = zpool.tile([1, TB], f32, tag="rec_s")
                            nc.vector.reciprocal(rec_s, combo[0:1, :])
                            rec_a = zpool.tile([1, TB], f32, tag="rec_a")
                            nc.vector.reciprocal(rec_a, combo[64:65, :])
                            fac = zpool.tile([1, TB], f32, tag="fac")
                            nc.vector.tensor_mul(fac, combo[0:1, :], rec_a)
                            fac10 = zpool.tile([AL, TB], f32, tag="fac10")
                            nc.gpsimd.partition_broadcast(fac10, fac)
                            aes = apool.tile([AL, TB], bf16, tag="aes")
                            nc.vector.tensor_mul(aes, ae2, fac10)
                            rs128 = zpool.tile([128, TB], f32, tag="rs128")
                            nc.gpsimd.partition_broadcast(rs128, rec_s)
                            po2 = ops.tile([128, TB], f32, tag="o")
                            for i, k in enumerate(kept):
                                nc.tensor.matmul(
                                    po2, vasb[:, k, h, :], ptsb[:, k, :],
                                    start=(i == 0), stop=False)
                            nc.tensor.matmul(
                                po2, avsb[b][:, h * 128:(h + 1) * 128], aes,
                                start=False, stop=True)
                            osb = zpool.tile([128, TB], bf16, tag="osb")
                            nc.vector.tensor_mul(osb, po2, rs128)
                            for half in range(2):
                                j = 2 * qr + half
                                nc.gpsimd.dma_start(
                                    a2a_in[b][j * LD + h * 128:
                                              j * LD + (h + 1) * 128, :],
                                    osb[:, half * TPC:(half + 1) * TPC])

                        prev = None
                        for i, u in enumerate(units):
                            if pre_unit is not None:
                                pre_unit(i)
                            cur = (u, *stage_scores(u))
                            if prev is not None:
                                stage_tail(*prev)
                                if post_unit is not None:
                                    post_unit(i - 1)
                            prev = cur
                        stage_tail(*prev)
                        if post_unit is not None:
                            post_unit(len(units) - 1)
                        nc.gpsimd.collective_compute(
                            "AllToAll", bass.mybir.AluOpType.bypass,
                            replica_groups=rg8,
                            ins=[a2a_in[b][:].opt()],
                            outs=[a2a_out[b][:].opt()])

                    # ------------- Phase 2: attention batch 0 + a2a0 --------
                    with tc.tile_pool(name="sps0", bufs=3, space="PSUM") as sps0, \
                         tc.tile_pool(name="ops0", bufs=2, space="PSUM") as ops0, \
                         tc.tile_pool(name="cps0", bufs=2, space="PSUM") as cps0, \
                         tc.tile_pool(name="ppool0", bufs=2) as ppool0, \
                         tc.tile_pool(name="apool0", bufs=2) as apool0, \
                         tc.tile_pool(name="zpool0", bufs=2) as zpool0:

                        def pre0(i):
                            if i == 0:
                                for h in range(HPC):
                                    load_v_head(0, h)

                        def post0(i):
                            # after head h's last unit, overwrite K/Q/V with
                            # batch-1 data (loads overlap remaining batch-0)
                            if i % NQR == NQR - 1:
                                load_b1_head(i // NQR)

                        attn_batch(0, units_h, sps0, ops0, cps0,
                                   ppool0, apool0, zpool0,
                                   pre_unit=pre0, post_unit=post0)

                    # -------- Phase 3: attention batch 1 + projection b0 ----
                    with tc.tile_pool(name="wopool", bufs=2) as wopool, \
                         tc.tile_pool(name="atpool", bufs=1) as atpool, \
                         tc.tile_pool(name="opool", bufs=4) as opool:
                        wore = wot[:].rearrange("(kc p) o -> p kc o", p=128)

                        def proj_batch(b, pps, oc_range, attsb_box):
                            if attsb_box[0] is None:
                                attsb_box[0] = atpool.tile(
                                    [128, KC, TPC], bf16, tag="att",
                                    name="attsb")
                                are = a2a_out[b][:].rearrange(
                                    "(kc p) t -> p kc t", p=128)
                                for kg in range(4):
                                    nc.sync.dma_start(
                                        attsb_box[0][:, kg * 8:(kg + 1) * 8, :],
                                        are[:, kg * 8:(kg + 1) * 8, :])
                            attsb = attsb_box[0]
                            for oc in oc_range:
                                wosb = wopool.tile([128, KC, TB], bf16, tag="wo")
                                nc.sync.dma_start(
                                    wosb, wore[:, :, oc * TB:(oc + 1) * TB])
                                for tblk in range(TPC // 128):
                                    pp = pps.tile([128, TB], f32, tag="pp")
                                    for k in range(KC):
                                        nc.tensor.matmul(
                                            pp,
                                            attsb[:, k, tblk * 128:(tblk + 1) * 128],
                                            wosb[:, k, :],
                                            start=(k == 0), stop=(k == KC - 1))
                                    psb = opool.tile([128, TB], f32, tag="ps")
                                    if (oc + tblk) % 2 == 0:
                                        nc.scalar.activation(psb, pp, AF.Copy)
                                    else:
                                        nc.vector.tensor_copy(psb, pp)
                                    nc.scalar.dma_start(
                                        out[b * TPC + tblk * 128:
                                            b * TPC + (tblk + 1) * 128,
                                            oc * TB:(oc + 1) * TB], psb)

                        with tc.tile_pool(name="sps1", bufs=2, space="PSUM") as sps1, \
                             tc.tile_pool(name="ops1", bufs=2, space="PSUM") as ops1, \
                             tc.tile_pool(name="cps1", bufs=2, space="PSUM") as cps1, \
                             tc.tile_pool(name="pps0", bufs=2, space="PSUM") as pps0, \
                             tc.tile_pool(name="ppool1", bufs=2) as ppool1, \
                             tc.tile_pool(name="apool1", bufs=2) as apool1, \
                             tc.tile_pool(name="zpool1", bufs=2) as zpool1:
                            box0 = [None]

                            # proj-b0 oc j is emitted AHEAD of unit (7+j)'s
                            # scores: by unit 7 the a2a0 collective has
                            # landed, and leading with proj keeps the PE fed
                            # while the scalar engine works through exps
                            def pre1(i):
                                if 7 <= i <= 14:
                                    proj_batch(0, pps0, [i - 7], box0)

                            attn_batch(1, units_q, sps1, ops1, cps1,
                                       ppool1, apool1, zpool1,
                                       pre_unit=pre1)

                        # ---------------- Phase 4: projection batch 1 -------
                        with tc.tile_pool(name="pps1", bufs=2, space="PSUM") as pps1:
                            proj_batch(1, pps1, list(range(D // TB)), [None])

    nc.compile()
    return nc


def _prep_inputs(x, mask, adapter, wq, wk, wv, wo,
                 lora_q1, lora_q2, lora_k1, lora_k2, lora_v1, lora_v2,
                 lora_o1, lora_o2, gate, new_gate):
    """Host-side sharding: returns in_maps (list of 8 dicts)."""
    def bf(a):
        return np.ascontiguousarray(np.asarray(a, np.float32).astype(BF16))

    f32 = np.float32
    wq_eff = (np.asarray(wq, f32)
              + np.asarray(lora_q2, f32) @ np.asarray(lora_q1, f32)) * SCALE
    wk_eff = np.asarray(wk, f32) + np.asarray(lora_k2, f32) @ np.asarray(lora_k1, f32)
    wv_eff = np.asarray(wv, f32) + np.asarray(lora_v2, f32) @ np.asarray(lora_v1, f32)
    wo_eff = np.asarray(wo, f32) + np.asarray(lora_o2, f32) @ np.asarray(lora_o1, f32)

    x2 = np.asarray(x, f32).reshape(T, D)
    xt = bf(x2.T)
    wot = bf(wo_eff.T)
    maskt = bf(np.asarray(mask, f32)[0, 0].T)
    gf_all = (np.tanh(np.asarray(gate, f32)[0, :, 0, 0])
              * np.asarray(new_gate, f32)[0, 0, 0, 0])

    # adapter K/V with the plain wk/wv (reference applies no LoRA there);
    # adapter scores use the pre-scaled q, so no extra scale needed here.
    # tanh(gate)*new_gate is folded into the adapter V (per-head columns).
    a2 = np.asarray(adapter, f32)                       # [B, AL, D]
    ak_all = a2 @ np.asarray(wk, f32).T                 # [B, AL, D]
    av_all = a2 @ np.asarray(wv, f32).T
    avg_all = av_all * np.repeat(gf_all, HD)[None, None, :]

    in_maps = []
    for c in range(NCORES):
        sl = slice(c * LD, (c + 1) * LD)
        wqkvt = bf(np.concatenate([wq_eff[sl].T, wk_eff[sl].T, wv_eff[sl].T],
                                  axis=1))
        akt_np = np.zeros((128, HPC, B, AL), f32)
        for m in range(HPC):
            for b in range(B):
                akt_np[:, m, b, :] = ak_all[b, :, c * LD + m * 128:
                                            c * LD + (m + 1) * 128].T
        aktp = bf(akt_np.reshape(128, HPC * B * AL))
        avp = bf(avg_all[:, :, sl].reshape(B * AL, LD))
        in_maps.append({
            "xt": xt, "wqkvt": wqkvt, "wot": wot, "aktp": aktp,
            "avp": avp, "maskt": maskt,
        })
    return in_maps


def kernel(x, start_pos, freqs_cis, mask, adapter,
           wq, wk, wv, wo,
           lora_q1, lora_q2, lora_k1, lora_k2,
           lora_v1, lora_v2, lora_o1, lora_o2,
           gate, new_gate, _trace=False):
    from concourse.bass_utils import run_bass_kernel_spmd

    keep, need, mreps, mmap = _mask_pattern(mask)
    if _CACHE.get("pattern") != (keep, need, mreps, mmap):
        _CACHE["nc"] = _build(keep, need, mreps, mmap)
        _CACHE["pattern"] = (keep, need, mreps, mmap)
    nc = _CACHE["nc"]

    in_maps = _prep_inputs(x, mask, adapter, wq, wk, wv, wo,
                           lora_q1, lora_q2, lora_k1, lora_k2,
                           lora_v1, lora_v2, lora_o1, lora_o2, gate, new_gate)
    kw = {}
    if _trace:
        kw["tmpdir"] = "/tmp/ktrace"
        import os
        import shutil
        shutil.rmtree("/tmp/ktrace", ignore_errors=True)
        os.makedirs("/tmp/ktrace", exist_ok=True)
    res = run_bass_kernel_spmd(nc, in_maps, list(range(NCORES)), trace=_trace, **kw)
    _CACHE["last_exec_ns"] = res.exec_time_ns
    _CACHE["last_res"] = res
    outs = [np.asarray(res.results[c]["out"], np.float32) for c in range(NCORES)]
    # core c rows: [b*TPC:(b+1)*TPC] = batch b tokens [c*TPC:(c+1)*TPC]
    full = np.concatenate(
        [np.concatenate([o[b * TPC:(b + 1) * TPC] for o in outs], axis=0)
         for b in range(B)], axis=0).reshape(B, S, D)
    return full
